# revision 11
# baseline (speedup 1.0000x reference)
"""Trainium2 Bass kernel for BSQ (binary spherical quantization) codebook forward.

Math: out = sign(x @ W_enc.T + b_enc) @ W_dec.T + b_dec
(The L2-normalize in the reference is a forward no-op: dividing by a positive
norm never changes the sign, and the eps-clamped zero-vector case produces
sign(0)=+1 either way.)

Strategy (pure data parallel over 8 NeuronCores, 8192 tokens each):
- x is rounded to fp16 and transposed ON THE HOST into feature-major
  [chunk, 128, tokens] layout, so the device sees plain full-bandwidth DMA
  loads on the sync-engine HWDGE queue — no DMA x-bar transposes. fp16-only
  x flips the sign of ~55/65536 tokens vs fp32 (rel err 1.4e-2, under the
  2e-2 budget); the weight-side rounding is cancelled exactly by the
  xh@Wh + xh@Wl hi/lo product pair (no extra DMA, 4 extra matmul waves).
- mm1: z.T per 512-token subtile accumulated in PSUM from 8 fp16 matmuls
  (2 weight products x 4 K-chunks). The 4 subtiles of each 2048-token
  block run in 4 distinct PE column strips (tile_position=(0,32s)) and
  pack ~4.7x concurrent. Each weight group is padded to 32 columns
  (16..31 zero) so all 128 z rows are written and a SINGLE DVE is_ge per
  block computes q.
- sign: one tensor_scalar is_ge per block against a per-partition
  threshold: -b_enc on the 16 real rows of each 32-row band, -1 on the
  rest (0 >= -1 -> 1.0 gives the "+1" bias row for free; rows 17-31 are
  junk 1.0s that nothing reads).
- mm2: out[128,512] = q_aug[17,:].T @ [2*W_dec.T ; b_dec - W_dec.sum(1)],
  one matmul per 128 tokens, row-packed across subtiles
  (tile_position=(32s,0)), pairs of token-groups sharing a 2-bank PSUM
  tile so each PSUM->SBUF copy moves [128,1024].
- The mm2 pairs of block b are INTERLEAVED between the mm1 waves of
  block b+1 in the instruction stream: an isolated mm2 burst runs at the
  PSUM-drain-copy pace (~4.8us/block across DVE+ScalarE, the only two
  engines that can read PSUM) with the PE half-idle; interleaved, the
  copies drain in the shadow of mm1 and the PE stays saturated.
- The fp16 output DMAs ride the Act-engine HWDGE queue (sharing the sync
  queue would serialize behind the input stream: a queue stripes every
  DMA across its 16 hw engines strictly in order). The host upcasts the
  fp16 output to fp32 (costs 2e-4 rel err on top).
"""

import numpy as np

import concourse.bacc as bacc
import concourse.mybir as mybir
from concourse import tile
from concourse.bass_utils import run_bass_kernel_spmd

NCORES = 8
B, H, W_, D = 64, 32, 32, 512
C = 16            # codebook bits
CA = C + 1        # + the constant-one row for the decoder bias
P = 128           # partitions
NCH = D // P      # 4 K-chunks for the encoder contraction
TOK = (B // NCORES) * H * W_   # 8192 tokens per core
BLK = 2048        # tokens per z/output block
SUB = 512         # tokens per z subtile (one PSUM accumulation group)
NSUB = BLK // SUB  # 4 subtiles = 4 PE column/row strips
NBLK = TOK // BLK  # 4 blocks
MW = 32           # padded columns per w1 product group (17 real)
NW1 = 2 * NCH * MW  # 256 w1 columns: (Wh, Wl) x 4 chunks x 32

_CACHE = {}


def _build_nc():
    f16, f32 = mybir.dt.float16, mybir.dt.float32
    nc = bacc.Bacc(
        "TRN2",
        target_bir_lowering=False,
        debug=False,
        enable_asserts=False,
        num_devices=NCORES,
    )
    xt = nc.dram_tensor("xt", [NCH, P, TOK], f16, kind="ExternalInput").ap()
    w1 = nc.dram_tensor("w1", [P, NW1], f16, kind="ExternalInput").ap()
    w2 = nc.dram_tensor("w2", [P, D], f16, kind="ExternalInput").ap()
    nb = nc.dram_tensor("nb", [P, 1], f32, kind="ExternalInput").ap()
    out = nc.dram_tensor("out", [P, TOK // P, D], f16, kind="ExternalOutput").ap()

    with tile.TileContext(nc) as tc:
        with (
            tc.tile_pool(name="consts", bufs=1) as cpool,
            tc.tile_pool(name="xt", bufs=NCH * NBLK) as xpool,
            tc.tile_pool(name="q", bufs=2) as qpool,
            tc.tile_pool(name="osb", bufs=NBLK * NSUB) as opool,
            tc.tile_pool(name="zps", bufs=2, space="PSUM") as zpool,
            tc.tile_pool(name="ops", bufs=3, space="PSUM") as opspool,
        ):
            # Small weights ride the (otherwise idle-at-start) Act queue.
            w1_sb = cpool.tile([P, NW1], f16)
            nc.scalar.dma_start(out=w1_sb[:], in_=w1)
            w2_sb = cpool.tile([P, D], f16)
            nc.scalar.dma_start(out=w2_sb[:], in_=w2)
            negb_sb = cpool.tile([P, 1], f32)
            nc.scalar.dma_start(out=negb_sb[:], in_=nb)

            # Fully-resident transposed x, one plain DMA per (chunk, block)
            # on the sync-engine queue so each block's compute unlocks as
            # its 4 chunk slices land.
            x_cb = [
                [xpool.tile([P, BLK], f16, tag="xt", name=f"x{c}b{b}") for b in range(NBLK)]
                for c in range(NCH)
            ]
            for b in range(NBLK):
                for c in range(NCH):
                    nc.sync.dma_start(
                        out=x_cb[c][b][:],
                        in_=xt[c, :, b * BLK:(b + 1) * BLK],
                    )

            z_ps = [zpool.tile([P, SUB], f32, tag="z", name=f"z{b}") for b in range(NBLK)]
            q_sbs = {}
            o_sbs = {}

            def mm1_wave(b, i):
                ci, p = i // 2, i % 2
                wofs = (p * NCH + ci) * MW
                for s in range(NSUB):
                    nc.tensor.matmul(
                        z_ps[b][32 * s:32 * s + MW, :],
                        w1_sb[:, wofs:wofs + MW],
                        x_cb[ci][b][:, s * SUB:(s + 1) * SUB],
                        start=(i == 0),
                        stop=(i == 2 * NCH - 1),
                        tile_position=(0, 32 * s),
                        skip_group_check=True,
                    )

            def emit_sign(b):
                q_sb = qpool.tile([P, SUB], f16, tag="q", name=f"q{b}")
                nc.vector.tensor_scalar(
                    out=q_sb[:],
                    in0=z_ps[b][:],
                    scalar1=negb_sb[:],
                    scalar2=None,
                    op0=mybir.AluOpType.is_ge,
                )
                q_sbs[b] = q_sb
                o_sbs[b] = [
                    opool.tile([P, NSUB * D], f16, tag="osb", name=f"osb{b}_{s}")
                    for s in range(NSUB)
                ]

            def mm2_pair(b, i):
                s, gp = i // 2, i % 2
                q_sb = q_sbs[b]
                o_ps = opspool.tile([P, 2 * D], f32, tag="ops", name=f"ops{b}_{s}_{gp}")
                for gi in range(2):
                    g = 2 * gp + gi
                    nc.tensor.matmul(
                        o_ps[:, gi * D:(gi + 1) * D],
                        q_sb[32 * s:32 * s + CA, g * P:(g + 1) * P],
                        w2_sb[32 * s:32 * s + CA, :],
                        start=True,
                        stop=True,
                        tile_position=(32 * s, 0),
                        skip_group_check=True,
                    )
                # GpSimd cannot read PSUM: split the fp32->fp16 drain
                # copies evenly between DVE and ScalarE.
                dst = o_sbs[b][s][:, gp * 2 * D:(gp + 1) * 2 * D]
                if i % 2 == 0:
                    nc.scalar.copy(out=dst, in_=o_ps[:])
                else:
                    nc.vector.tensor_copy(out=dst, in_=o_ps[:])
                if gp == 1:
                    g0 = (b * BLK + s * SUB) // P
                    nc.scalar.dma_start(
                        out=out[:, g0:g0 + NSUB, :],
                        in_=o_sbs[b][s][:],
                    )

            # Software pipeline: mm1 waves of block b+1 are interleaved
            # with the mm2 pairs of block b.
            for i in range(2 * NCH):
                mm1_wave(0, i)
            emit_sign(0)
            for b in range(NBLK):
                for i in range(2 * NCH):
                    if b + 1 < NBLK:
                        mm1_wave(b + 1, i)
                    mm2_pair(b, i)
                if b + 1 < NBLK:
                    emit_sign(b + 1)
    nc.compile()
    return nc


def _get_nc():
    if "nc" not in _CACHE:
        _CACHE["nc"] = _build_nc()
    return _CACHE["nc"]


def _prep_weights(W_enc, b_enc, W_dec, b_dec):
    f16, f32 = np.float16, np.float32
    WT = np.ascontiguousarray(W_enc.T.astype(f32))            # [512, 16]
    Wh = WT.astype(f16)
    Wl = (WT - Wh.astype(f32)).astype(f16)
    # 8 lhsT tiles of [128, 32]: (Wh, Wl) per K-chunk, cols 16..31 = 0 so
    # every z row is written (row 16 = 0 feeds the bias trick, 17..31 junk)
    w1 = np.zeros((P, NW1), f16)
    for p, src in enumerate((Wh, Wl)):
        for c in range(NCH):
            ofs = (p * NCH + c) * MW
            w1[:, ofs:ofs + C] = src[c * P:(c + 1) * P, :]

    # w2: replica of [2*W_dec.T ; bias_row] in each 32-row band; nb: the
    # per-partition sign thresholds (-b_enc on the 16 real rows, -1
    # elsewhere: the zero z bias-row maps to q=1, rows 17..31 are unread).
    w2 = np.zeros((P, D), f16)
    band = np.concatenate(
        [2.0 * W_dec.T.astype(f32),
         (b_dec.astype(f32) - W_dec.astype(f32).sum(axis=1)).reshape(1, D)],
        axis=0,
    ).astype(f16)                                             # [17, 512]
    negb = np.full((P, 1), -1.0, f32)
    for s in range(NSUB):
        w2[32 * s:32 * s + CA, :] = band
        negb[32 * s:32 * s + C, 0] = -b_enc.astype(f32)
    return w1, w2, negb


def _prep_x_shard(x_flat_shard):
    """[8192, 512] fp32 -> [4, 128, 8192] fp16 feature-major (chunk, part, tok)."""
    xh = x_flat_shard.astype(np.float16)
    return np.ascontiguousarray(xh.T).reshape(NCH, P, TOK)


def kernel(x, W_enc, b_enc, W_dec, b_dec, _trace=False, _trace_kwargs=None):
    x = np.asarray(x, dtype=np.float32)
    w1, w2, nb = _prep_weights(
        np.asarray(W_enc), np.asarray(b_enc), np.asarray(W_dec), np.asarray(b_dec)
    )
    xf = x.reshape(NCORES, TOK, D)
    in_maps = []
    for s in range(NCORES):
        in_maps.append(dict(xt=_prep_x_shard(xf[s]), w1=w1, w2=w2, nb=nb))
    nc = _get_nc()
    res = run_bass_kernel_spmd(
        nc,
        in_maps,
        core_ids=list(range(NCORES)),
        trace=_trace,
        **(_trace_kwargs or {}),
    )
    out = np.concatenate(
        [
            res.results[s]["out"].transpose(1, 0, 2).reshape(1, TOK, D)
            for s in range(NCORES)
        ],
        axis=0,
    ).astype(np.float32).reshape(B, H, W_, D)
    _CACHE["last_results"] = res
    return out


# revision 13
# speedup vs baseline: 1.0663x; 1.0663x over previous
"""Trainium2 Bass kernel for BSQ (binary spherical quantization) codebook forward.

Math: out = sign(x @ W_enc.T + b_enc) @ W_dec.T + b_dec
(The L2-normalize in the reference is a forward no-op: dividing by a positive
norm never changes the sign, and the eps-clamped zero-vector case produces
sign(0)=+1 either way.)

Strategy (pure data parallel over 8 NeuronCores, 8192 tokens each):
- x is rounded to fp16 and transposed ON THE HOST into feature-major
  [chunk, 128, tokens] layout, so the device sees plain full-bandwidth DMA
  loads on the sync-engine HWDGE queue — no DMA x-bar transposes. fp16-only
  x flips the sign of ~55/65536 tokens vs fp32 (rel err 1.4e-2, under the
  2e-2 budget); the weight-side rounding is cancelled exactly by the
  xh@Wh + xh@Wl hi/lo product pair (no extra DMA, 4 extra matmul waves).
- mm1: z.T per 512-token subtile accumulated in PSUM from 8 fp16 matmuls
  (2 weight products x 4 K-chunks). The 4 subtiles of each 2048-token
  block run in 4 distinct PE column strips (tile_position=(0,32s)) and
  pack ~4.7x concurrent. Each weight group is padded to 32 columns
  (16..31 zero) so all 128 z rows are written and a SINGLE DVE is_ge per
  block computes q.
- sign: one tensor_scalar is_ge per block against a per-partition
  threshold: -b_enc on the 16 real rows of each 32-row band, -1 on the
  rest (0 >= -1 -> 1.0 gives the "+1" bias row for free; rows 17-31 are
  junk 1.0s that nothing reads).
- mm2: out[128,512] = q_aug[17,:].T @ [2*W_dec.T ; b_dec - W_dec.sum(1)],
  one matmul per 128 tokens, row-packed across subtiles
  (tile_position=(32s,0)), pairs of token-groups sharing a 2-bank PSUM
  tile so each PSUM->SBUF copy moves [128,1024].
- The mm2 pairs of block b are INTERLEAVED between the mm1 waves of
  block b+1 in the instruction stream: an isolated mm2 burst runs at the
  PSUM-drain-copy pace (~4.8us/block across DVE+ScalarE, the only two
  engines that can read PSUM) with the PE half-idle; interleaved, the
  copies drain in the shadow of mm1 and the PE stays saturated.
- The fp16 output DMAs ride the Act-engine HWDGE queue (sharing the sync
  queue would serialize behind the input stream: a queue stripes every
  DMA across its 16 hw engines strictly in order). The host upcasts the
  fp16 output to fp32 (costs 2e-4 rel err on top).
"""

import numpy as np

import concourse.bacc as bacc
import concourse.mybir as mybir
from concourse import tile
from concourse.bass_utils import run_bass_kernel_spmd

NCORES = 8
B, H, W_, D = 64, 32, 32, 512
C = 16            # codebook bits
CA = C + 1        # + the constant-one row for the decoder bias
P = 128           # partitions
NCH = D // P      # 4 K-chunks for the encoder contraction
TOK = (B // NCORES) * H * W_   # 8192 tokens per core
BLK = 2048        # tokens per z/output block
SUB = 512         # tokens per z subtile (one PSUM accumulation group)
NSUB = BLK // SUB  # 4 subtiles = 4 PE column/row strips
NBLK = TOK // BLK  # 4 blocks
MW = 32           # padded columns per w1 product group (17 real)
NW1 = 2 * NCH * MW  # 256 w1 columns: (Wh, Wl) x 4 chunks x 32

_CACHE = {}


def _build_nc():
    f16, f32 = mybir.dt.float16, mybir.dt.float32
    nc = bacc.Bacc(
        "TRN2",
        target_bir_lowering=False,
        debug=False,
        enable_asserts=False,
        num_devices=NCORES,
    )
    xt = nc.dram_tensor("xt", [NCH, P, TOK], f16, kind="ExternalInput").ap()
    w1 = nc.dram_tensor("w1", [P, NW1], f16, kind="ExternalInput").ap()
    w2 = nc.dram_tensor("w2", [P, D], f16, kind="ExternalInput").ap()
    nb = nc.dram_tensor("nb", [P, 1], f32, kind="ExternalInput").ap()
    out = nc.dram_tensor("out", [P, TOK // P, D], f16, kind="ExternalOutput").ap()

    with tile.TileContext(nc) as tc:
        with (
            tc.tile_pool(name="consts", bufs=1) as cpool,
            tc.tile_pool(name="xt", bufs=NCH * NBLK) as xpool,
            tc.tile_pool(name="q", bufs=2) as qpool,
            tc.tile_pool(name="osb", bufs=NBLK * NSUB) as opool,
            tc.tile_pool(name="zps", bufs=2, space="PSUM") as zpool,
            tc.tile_pool(name="ops", bufs=3, space="PSUM") as opspool,
        ):
            # Small weights ride the (otherwise idle-at-start) Act queue.
            w1_sb = cpool.tile([P, NW1], f16)
            nc.scalar.dma_start(out=w1_sb[:], in_=w1)
            w2_sb = cpool.tile([P, D], f16)
            nc.scalar.dma_start(out=w2_sb[:], in_=w2)
            negb_sb = cpool.tile([P, 1], f32)
            nc.scalar.dma_start(out=negb_sb[:], in_=nb)

            # Fully-resident transposed x, one plain DMA per (chunk, block)
            # on the sync-engine queue so each block's compute unlocks as
            # its 4 chunk slices land.
            x_cb = [
                [xpool.tile([P, BLK], f16, tag="xt", name=f"x{c}b{b}") for b in range(NBLK)]
                for c in range(NCH)
            ]
            for b in range(NBLK):
                for c in range(NCH):
                    nc.sync.dma_start(
                        out=x_cb[c][b][:],
                        in_=xt[c, :, b * BLK:(b + 1) * BLK],
                    )

            z_ps = [zpool.tile([P, SUB], f32, tag="z", name=f"z{b}") for b in range(NBLK)]
            q_sbs = {}
            o_sbs = {}

            def mm1_wave(b, i):
                ci, p = i // 2, i % 2
                wofs = (p * NCH + ci) * MW
                for s in range(NSUB):
                    nc.tensor.matmul(
                        z_ps[b][32 * s:32 * s + MW, :],
                        w1_sb[:, wofs:wofs + MW],
                        x_cb[ci][b][:, s * SUB:(s + 1) * SUB],
                        start=(i == 0),
                        stop=(i == 2 * NCH - 1),
                        tile_position=(0, 32 * s),
                        skip_group_check=True,
                    )

            def emit_sign(b):
                q_sb = qpool.tile([P, SUB], f16, tag="q", name=f"q{b}")
                nc.vector.tensor_scalar(
                    out=q_sb[:],
                    in0=z_ps[b][:],
                    scalar1=negb_sb[:],
                    scalar2=None,
                    op0=mybir.AluOpType.is_ge,
                )
                q_sbs[b] = q_sb
                o_sbs[b] = [
                    opool.tile([P, NSUB * D], f16, tag="osb", name=f"osb{b}_{s}")
                    for s in range(NSUB)
                ]

            def mm2_pair(b, i):
                s, gp = i // 2, i % 2
                q_sb = q_sbs[b]
                o_ps = opspool.tile([P, 2 * D], f32, tag="ops", name=f"ops{b}_{s}_{gp}")
                for gi in range(2):
                    g = 2 * gp + gi
                    nc.tensor.matmul(
                        o_ps[:, gi * D:(gi + 1) * D],
                        q_sb[32 * s:32 * s + CA, g * P:(g + 1) * P],
                        w2_sb[32 * s:32 * s + CA, :],
                        start=True,
                        stop=True,
                        tile_position=(32 * s, 0),
                        skip_group_check=True,
                    )
                # GpSimd cannot read PSUM: split the fp32->fp16 drain
                # copies evenly between DVE and ScalarE.
                dst = o_sbs[b][s][:, gp * 2 * D:(gp + 1) * 2 * D]
                if i % 2 == 0:
                    nc.scalar.copy(out=dst, in_=o_ps[:])
                else:
                    nc.vector.tensor_copy(out=dst, in_=o_ps[:])
                if gp == 1:
                    # Output DMAs ride the GpSimd software-DGE queue: its
                    # issue cost lands on the otherwise-idle Pool engine and
                    # the stream does not share a HWDGE ring with the input
                    # loads (a queue drains its DMAs strictly in order).
                    g0 = (b * BLK + s * SUB) // P
                    nc.gpsimd.dma_start(
                        out=out[:, g0:g0 + NSUB, :],
                        in_=o_sbs[b][s][:],
                    )

            # Software pipeline: emit the whole mm1 of block b+1 before the
            # mm2 pairs of block b, so the PE rolls into the next block's
            # encoder while the DVE sign op for block b completes. (Finer
            # per-wave interleaving of mm1 and mm2 was measured WORSE —
            # alternating the two PE tile configurations kills the
            # column/row-strip stream packing.)
            for i in range(2 * NCH):
                mm1_wave(0, i)
            for b in range(NBLK):
                emit_sign(b)
                if b + 1 < NBLK:
                    for i in range(2 * NCH):
                        mm1_wave(b + 1, i)
                for i in range(2 * NCH):
                    mm2_pair(b, i)
    nc.compile()
    return nc


def _get_nc():
    if "nc" not in _CACHE:
        _CACHE["nc"] = _build_nc()
    return _CACHE["nc"]


def _prep_weights(W_enc, b_enc, W_dec, b_dec):
    f16, f32 = np.float16, np.float32
    WT = np.ascontiguousarray(W_enc.T.astype(f32))            # [512, 16]
    Wh = WT.astype(f16)
    Wl = (WT - Wh.astype(f32)).astype(f16)
    # 8 lhsT tiles of [128, 32]: (Wh, Wl) per K-chunk, cols 16..31 = 0 so
    # every z row is written (row 16 = 0 feeds the bias trick, 17..31 junk)
    w1 = np.zeros((P, NW1), f16)
    for p, src in enumerate((Wh, Wl)):
        for c in range(NCH):
            ofs = (p * NCH + c) * MW
            w1[:, ofs:ofs + C] = src[c * P:(c + 1) * P, :]

    # w2: replica of [2*W_dec.T ; bias_row] in each 32-row band; nb: the
    # per-partition sign thresholds (-b_enc on the 16 real rows, -1
    # elsewhere: the zero z bias-row maps to q=1, rows 17..31 are unread).
    w2 = np.zeros((P, D), f16)
    band = np.concatenate(
        [2.0 * W_dec.T.astype(f32),
         (b_dec.astype(f32) - W_dec.astype(f32).sum(axis=1)).reshape(1, D)],
        axis=0,
    ).astype(f16)                                             # [17, 512]
    negb = np.full((P, 1), -1.0, f32)
    for s in range(NSUB):
        w2[32 * s:32 * s + CA, :] = band
        negb[32 * s:32 * s + C, 0] = -b_enc.astype(f32)
    return w1, w2, negb


def _prep_x_shard(x_flat_shard):
    """[8192, 512] fp32 -> [4, 128, 8192] fp16 feature-major (chunk, part, tok)."""
    xh = x_flat_shard.astype(np.float16)
    return np.ascontiguousarray(xh.T).reshape(NCH, P, TOK)


def kernel(x, W_enc, b_enc, W_dec, b_dec, _trace=False, _trace_kwargs=None):
    x = np.asarray(x, dtype=np.float32)
    w1, w2, nb = _prep_weights(
        np.asarray(W_enc), np.asarray(b_enc), np.asarray(W_dec), np.asarray(b_dec)
    )
    xf = x.reshape(NCORES, TOK, D)
    in_maps = []
    for s in range(NCORES):
        in_maps.append(dict(xt=_prep_x_shard(xf[s]), w1=w1, w2=w2, nb=nb))
    nc = _get_nc()
    res = run_bass_kernel_spmd(
        nc,
        in_maps,
        core_ids=list(range(NCORES)),
        trace=_trace,
        **(_trace_kwargs or {}),
    )
    out = np.concatenate(
        [
            res.results[s]["out"].transpose(1, 0, 2).reshape(1, TOK, D)
            for s in range(NCORES)
        ],
        axis=0,
    ).astype(np.float32).reshape(B, H, W_, D)
    _CACHE["last_results"] = res
    return out


# revision 16
# speedup vs baseline: 1.2183x; 1.1426x over previous
"""Trainium2 Bass kernel for BSQ (binary spherical quantization) codebook forward.

Math: out = sign(x @ W_enc.T + b_enc) @ W_dec.T + b_dec
(The L2-normalize in the reference is a forward no-op: dividing by a positive
norm never changes the sign, and the eps-clamped zero-vector case produces
sign(0)=+1 either way.)

Strategy (pure data parallel over 8 NeuronCores, 8192 tokens each):
- x is rounded to fp16 and transposed ON THE HOST into feature-major
  [chunk, 128, tokens] layout, so the device sees plain full-bandwidth DMA
  loads on the sync-engine HWDGE queue — no DMA x-bar transposes. fp16-only
  x flips the sign of ~55/65536 tokens vs fp32 (rel err 1.4e-2, under the
  2e-2 budget); the weight-side rounding is cancelled exactly by the
  xh@Wh + xh@Wl hi/lo product pair (no extra DMA, 4 extra matmul waves).
- mm1: z.T per 512-token subtile accumulated in PSUM from 8 fp16 matmuls
  (2 weight products x 4 K-chunks). The 4 subtiles of each 2048-token
  block run in 4 distinct PE column strips (tile_position=(0,32s)) and
  pack ~4.7x concurrent. Each weight group is padded to 32 columns
  (16..31 zero) so all 128 z rows are written and a SINGLE DVE is_ge per
  block computes q.
- sign: one tensor_scalar is_ge per block against a per-partition
  threshold: -b_enc on the 16 real rows of each 32-row band, -1 on the
  rest (0 >= -1 -> 1.0 gives the "+1" bias row for free; rows 17-31 are
  junk 1.0s that nothing reads).
- mm2: out[128,512] = q_aug[17,:].T @ [2*W_dec.T ; b_dec - W_dec.sum(1)],
  one matmul per 128 tokens, row-packed across subtiles
  (tile_position=(32s,0)), pairs of token-groups sharing a 2-bank PSUM
  tile so each PSUM->SBUF copy moves [128,1024].
- The mm2 pairs of block b are INTERLEAVED between the mm1 waves of
  block b+1 in the instruction stream: an isolated mm2 burst runs at the
  PSUM-drain-copy pace (~4.8us/block across DVE+ScalarE, the only two
  engines that can read PSUM) with the PE half-idle; interleaved, the
  copies drain in the shadow of mm1 and the PE stays saturated.
- The fp16 output DMAs ride the Act-engine HWDGE queue (sharing the sync
  queue would serialize behind the input stream: a queue stripes every
  DMA across its 16 hw engines strictly in order). The host upcasts the
  fp16 output to fp32 (costs 2e-4 rel err on top).
"""

import numpy as np

import concourse.bacc as bacc
import concourse.mybir as mybir
from concourse import tile
from concourse.bass_utils import run_bass_kernel_spmd

NCORES = 8
B, H, W_, D = 64, 32, 32, 512
C = 16            # codebook bits
CA = C + 1        # + the constant-one row for the decoder bias
P = 128           # partitions
NCH = D // P      # 4 K-chunks for the encoder contraction
TOK = (B // NCORES) * H * W_   # 8192 tokens per core
BLK = 2048        # tokens per z/output block
SUB = 512         # tokens per z subtile (one PSUM accumulation group)
NSUB = BLK // SUB  # 4 subtiles = 4 PE column/row strips
NBLK = TOK // BLK  # 4 blocks
MW = 32           # padded columns per w1 product group (17 real)
NW1 = 2 * NCH * MW  # 256 w1 columns: (Wh, Wl) x 4 chunks x 32

_CACHE = {}


def _build_nc():
    f16, f32 = mybir.dt.float16, mybir.dt.float32
    nc = bacc.Bacc(
        "TRN2",
        target_bir_lowering=False,
        debug=False,
        enable_asserts=False,
        num_devices=NCORES,
    )
    xt = nc.dram_tensor("xt", [NCH, P, TOK], f16, kind="ExternalInput").ap()
    w1 = nc.dram_tensor("w1", [P, NW1], f16, kind="ExternalInput").ap()
    w2 = nc.dram_tensor("w2", [P, D], f16, kind="ExternalInput").ap()
    nb = nc.dram_tensor("nb", [P, 1], f32, kind="ExternalInput").ap()
    out = nc.dram_tensor("out", [P, TOK // P, D], f16, kind="ExternalOutput").ap()

    with tile.TileContext(nc) as tc:
        with (
            tc.tile_pool(name="consts", bufs=1) as cpool,
            tc.tile_pool(name="xt", bufs=NCH * NBLK) as xpool,
            tc.tile_pool(name="q", bufs=2) as qpool,
            tc.tile_pool(name="osb", bufs=NBLK * NSUB) as opool,
            tc.tile_pool(name="zps", bufs=2, space="PSUM") as zpool,
            tc.tile_pool(name="ops", bufs=3, space="PSUM") as opspool,
        ):
            # Small weights ride the (otherwise idle-at-start) Act queue.
            w1_sb = cpool.tile([P, NW1], f16)
            nc.scalar.dma_start(out=w1_sb[:], in_=w1)
            w2_sb = cpool.tile([P, D], f16)
            nc.scalar.dma_start(out=w2_sb[:], in_=w2)
            negb_sb = cpool.tile([P, 1], f32)
            nc.scalar.dma_start(out=negb_sb[:], in_=nb)

            # Fully-resident transposed x, one plain DMA per (chunk, block)
            # on the sync-engine queue so each block's compute unlocks as
            # its 4 chunk slices land.
            x_cb = [
                [xpool.tile([P, BLK], f16, tag="xt", name=f"x{c}b{b}") for b in range(NBLK)]
                for c in range(NCH)
            ]
            for b in range(NBLK):
                for c in range(NCH):
                    nc.sync.dma_start(
                        out=x_cb[c][b][:],
                        in_=xt[c, :, b * BLK:(b + 1) * BLK],
                    )

            z_ps = [zpool.tile([P, SUB], f32, tag="z", name=f"z{b}") for b in range(NBLK)]
            q_sbs = {}
            o_sbs = {}

            def mm1_wave(b, i):
                ci, p = i // 2, i % 2
                wofs = (p * NCH + ci) * MW
                for s in range(NSUB):
                    nc.tensor.matmul(
                        z_ps[b][32 * s:32 * s + MW, :],
                        w1_sb[:, wofs:wofs + MW],
                        x_cb[ci][b][:, s * SUB:(s + 1) * SUB],
                        start=(i == 0),
                        stop=(i == 2 * NCH - 1),
                        tile_position=(0, 32 * s),
                        skip_group_check=True,
                    )

            def emit_sign(b):
                q_sb = qpool.tile([P, SUB], f16, tag="q", name=f"q{b}")
                nc.vector.tensor_scalar(
                    out=q_sb[:],
                    in0=z_ps[b][:],
                    scalar1=negb_sb[:],
                    scalar2=None,
                    op0=mybir.AluOpType.is_ge,
                )
                q_sbs[b] = q_sb
                o_sbs[b] = [
                    opool.tile([P, NSUB * D], f16, tag="osb", name=f"osb{b}_{s}")
                    for s in range(NSUB)
                ]

            def mm2_pair(b, i):
                s, gp = i // 2, i % 2
                q_sb = q_sbs[b]
                o_ps = opspool.tile([P, 2 * D], f32, tag="ops", name=f"ops{b}_{s}_{gp}")
                for gi in range(2):
                    g = 2 * gp + gi
                    nc.tensor.matmul(
                        o_ps[:, gi * D:(gi + 1) * D],
                        q_sb[32 * s:32 * s + CA, g * P:(g + 1) * P],
                        w2_sb[32 * s:32 * s + CA, :],
                        start=True,
                        stop=True,
                        tile_position=(32 * s, 0),
                        skip_group_check=True,
                    )
                # GpSimd cannot read PSUM: split the fp32->fp16 drain
                # copies evenly between DVE and ScalarE.
                dst = o_sbs[b][s][:, gp * 2 * D:(gp + 1) * 2 * D]
                if i % 2 == 0 or i == 7:
                    nc.scalar.copy(out=dst, in_=o_ps[:])
                else:
                    nc.vector.tensor_copy(out=dst, in_=o_ps[:])
                if gp == 1:
                    # Output DMAs ride the sync HWDGE queue behind the input
                    # loads. Measured better than the alternatives: the Act
                    # queue loads the ScalarE with issue costs it cannot
                    # spare, and the GpSimd software-DGE path is slower
                    # outright; the in-order trailing behind the (earlier,
                    # production-paced anyway) input stream costs less.
                    g0 = (b * BLK + s * SUB) // P
                    nc.sync.dma_start(
                        out=out[:, g0:g0 + NSUB, :],
                        in_=o_sbs[b][s][:],
                    )

            # Software pipeline: emit the whole mm1 of block b+1 before the
            # mm2 pairs of block b, so the PE rolls into the next block's
            # encoder while the DVE sign op for block b completes. (Finer
            # per-wave interleaving of mm1 and mm2 was measured WORSE —
            # alternating the two PE tile configurations kills the
            # column/row-strip stream packing.)
            for i in range(2 * NCH):
                mm1_wave(0, i)
            for b in range(NBLK):
                if b + 1 < NBLK:
                    for i in range(2 * NCH):
                        mm1_wave(b + 1, i)
                emit_sign(b)
                for i in range(2 * NCH):
                    mm2_pair(b, i)
    nc.compile()
    return nc


def _get_nc():
    if "nc" not in _CACHE:
        _CACHE["nc"] = _build_nc()
    return _CACHE["nc"]


def _prep_weights(W_enc, b_enc, W_dec, b_dec):
    f16, f32 = np.float16, np.float32
    WT = np.ascontiguousarray(W_enc.T.astype(f32))            # [512, 16]
    Wh = WT.astype(f16)
    Wl = (WT - Wh.astype(f32)).astype(f16)
    # 8 lhsT tiles of [128, 32]: (Wh, Wl) per K-chunk, cols 16..31 = 0 so
    # every z row is written (row 16 = 0 feeds the bias trick, 17..31 junk)
    w1 = np.zeros((P, NW1), f16)
    for p, src in enumerate((Wh, Wl)):
        for c in range(NCH):
            ofs = (p * NCH + c) * MW
            w1[:, ofs:ofs + C] = src[c * P:(c + 1) * P, :]

    # w2: replica of [2*W_dec.T ; bias_row] in each 32-row band; nb: the
    # per-partition sign thresholds (-b_enc on the 16 real rows, -1
    # elsewhere: the zero z bias-row maps to q=1, rows 17..31 are unread).
    w2 = np.zeros((P, D), f16)
    band = np.concatenate(
        [2.0 * W_dec.T.astype(f32),
         (b_dec.astype(f32) - W_dec.astype(f32).sum(axis=1)).reshape(1, D)],
        axis=0,
    ).astype(f16)                                             # [17, 512]
    negb = np.full((P, 1), -1.0, f32)
    for s in range(NSUB):
        w2[32 * s:32 * s + CA, :] = band
        negb[32 * s:32 * s + C, 0] = -b_enc.astype(f32)
    return w1, w2, negb


def _prep_x_shard(x_flat_shard):
    """[8192, 512] fp32 -> [4, 128, 8192] fp16 feature-major (chunk, part, tok)."""
    xh = x_flat_shard.astype(np.float16)
    return np.ascontiguousarray(xh.T).reshape(NCH, P, TOK)


def kernel(x, W_enc, b_enc, W_dec, b_dec, _trace=False, _trace_kwargs=None):
    x = np.asarray(x, dtype=np.float32)
    w1, w2, nb = _prep_weights(
        np.asarray(W_enc), np.asarray(b_enc), np.asarray(W_dec), np.asarray(b_dec)
    )
    xf = x.reshape(NCORES, TOK, D)
    in_maps = []
    for s in range(NCORES):
        in_maps.append(dict(xt=_prep_x_shard(xf[s]), w1=w1, w2=w2, nb=nb))
    nc = _get_nc()
    res = run_bass_kernel_spmd(
        nc,
        in_maps,
        core_ids=list(range(NCORES)),
        trace=_trace,
        **(_trace_kwargs or {}),
    )
    out = np.concatenate(
        [
            res.results[s]["out"].transpose(1, 0, 2).reshape(1, TOK, D)
            for s in range(NCORES)
        ],
        axis=0,
    ).astype(np.float32).reshape(B, H, W_, D)
    _CACHE["last_results"] = res
    return out


# revision 19
# speedup vs baseline: 1.2978x; 1.0652x over previous
"""Trainium2 Bass kernel for BSQ (binary spherical quantization) codebook forward.

Math: out = sign(x @ W_enc.T + b_enc) @ W_dec.T + b_dec
(The L2-normalize in the reference is a forward no-op: dividing by a positive
norm never changes the sign, and the eps-clamped zero-vector case produces
sign(0)=+1 either way.)

Strategy (pure data parallel over 8 NeuronCores, 8192 tokens each):
- x is rounded to fp16 and transposed ON THE HOST into feature-major
  [chunk, 128, tokens] layout, so the device sees plain full-bandwidth DMA
  loads on the sync-engine HWDGE queue — no DMA x-bar transposes. fp16-only
  x flips the sign of ~55/65536 tokens vs fp32 (rel err 1.4e-2, under the
  2e-2 budget); the weight-side rounding is cancelled exactly by the
  xh@Wh + xh@Wl hi/lo product pair (no extra DMA, 4 extra matmul waves).
- mm1: z.T per 512-token subtile accumulated in PSUM from 8 fp16 matmuls
  (2 weight products x 4 K-chunks). The 4 subtiles of each 2048-token
  block run in 4 distinct PE column strips (tile_position=(0,32s)) and
  pack ~4.7x concurrent. Each weight group is padded to 32 columns
  (16..31 zero) so all 128 z rows are written and a SINGLE DVE is_ge per
  block computes q.
- sign: one tensor_scalar is_ge per block against a per-partition
  threshold: -b_enc on the 16 real rows of each 32-row band, -1 on the
  rest (0 >= -1 -> 1.0 gives the "+1" bias row for free; rows 17-31 are
  junk 1.0s that nothing reads).
- mm2: out[128,512] = q_aug[17,:].T @ [2*W_dec.T ; b_dec - W_dec.sum(1)],
  one matmul per 128 tokens, row-packed across subtiles
  (tile_position=(32s,0)), pairs of token-groups sharing a 2-bank PSUM
  tile so each PSUM->SBUF copy moves [128,1024].
- The mm2 pairs of block b are INTERLEAVED between the mm1 waves of
  block b+1 in the instruction stream: an isolated mm2 burst runs at the
  PSUM-drain-copy pace (~4.8us/block across DVE+ScalarE, the only two
  engines that can read PSUM) with the PE half-idle; interleaved, the
  copies drain in the shadow of mm1 and the PE stays saturated.
- The fp16 output DMAs ride the Act-engine HWDGE queue (sharing the sync
  queue would serialize behind the input stream: a queue stripes every
  DMA across its 16 hw engines strictly in order). The host upcasts the
  fp16 output to fp32 (costs 2e-4 rel err on top).
"""

import numpy as np

import concourse.bacc as bacc
import concourse.mybir as mybir
from concourse import tile
from concourse.bass_utils import run_bass_kernel_spmd

NCORES = 8
B, H, W_, D = 64, 32, 32, 512
C = 16            # codebook bits
CA = C + 1        # + the constant-one row for the decoder bias
P = 128           # partitions
NCH = D // P      # 4 K-chunks for the encoder contraction
TOK = (B // NCORES) * H * W_   # 8192 tokens per core
BLK = 2048        # tokens per z/output block
SUB = 512         # tokens per z subtile (one PSUM accumulation group)
NSUB = BLK // SUB  # 4 subtiles = 4 PE column/row strips
NBLK = TOK // BLK  # 4 blocks
MW = 32           # padded columns per w1 product group (17 real)
NW1 = 2 * NCH * MW  # 256 w1 columns: (Wh, Wl) x 4 chunks x 32

_CACHE = {}


def _build_nc():
    f16, f32 = mybir.dt.float16, mybir.dt.float32
    nc = bacc.Bacc(
        "TRN2",
        target_bir_lowering=False,
        debug=False,
        enable_asserts=False,
        num_devices=NCORES,
    )
    xt = nc.dram_tensor("xt", [NCH, P, TOK], f16, kind="ExternalInput").ap()
    w1 = nc.dram_tensor("w1", [P, NW1], f16, kind="ExternalInput").ap()
    w2 = nc.dram_tensor("w2", [P, D], f16, kind="ExternalInput").ap()
    nb = nc.dram_tensor("nb", [P, 1], f32, kind="ExternalInput").ap()
    out = nc.dram_tensor("out", [P, TOK // P, D], f16, kind="ExternalOutput").ap()

    with tile.TileContext(nc) as tc:
        with (
            tc.tile_pool(name="consts", bufs=1) as cpool,
            tc.tile_pool(name="xt", bufs=NCH * NBLK) as xpool,
            tc.tile_pool(name="q", bufs=2) as qpool,
            tc.tile_pool(name="osb", bufs=NBLK * NSUB) as opool,
            # PSUM: 1 bank for z (its reuse dep — mm1(b+1) after sign(b) —
            # is hidden behind the input stream pacing) and SEVEN 1-bank
            # slots for the mm2 drains: with only 3 slots the mm2 burst
            # stalls on the PSUM->SBUF copies and, the PE queue being
            # in-order, head-of-line blocks the next block's mm1 behind it
            # (~9.1us/block cadence instead of ~6.8).
            tc.tile_pool(name="zps", bufs=1, space="PSUM") as zpool,
            tc.tile_pool(name="ops", bufs=7, space="PSUM") as opspool,
        ):
            # Small weights ride the (otherwise idle-at-start) Act queue.
            w1_sb = cpool.tile([P, NW1], f16)
            nc.scalar.dma_start(out=w1_sb[:], in_=w1)
            w2_sb = cpool.tile([P, D], f16)
            nc.scalar.dma_start(out=w2_sb[:], in_=w2)
            negb_sb = cpool.tile([P, 1], f32)
            nc.scalar.dma_start(out=negb_sb[:], in_=nb)

            # Fully-resident transposed x, one plain DMA per (chunk, block)
            # on the sync-engine queue so each block's compute unlocks as
            # its 4 chunk slices land.
            x_cb = [
                [xpool.tile([P, BLK], f16, tag="xt", name=f"x{c}b{b}") for b in range(NBLK)]
                for c in range(NCH)
            ]
            for b in range(NBLK):
                for c in range(NCH):
                    nc.sync.dma_start(
                        out=x_cb[c][b][:],
                        in_=xt[c, :, b * BLK:(b + 1) * BLK],
                    )

            z_ps = [zpool.tile([P, SUB], f32, tag="z", name=f"z{b}") for b in range(NBLK)]
            q_sbs = {}
            o_sbs = {}

            def mm1_wave(b, i):
                ci, p = i // 2, i % 2
                wofs = (p * NCH + ci) * MW
                for s in range(NSUB):
                    nc.tensor.matmul(
                        z_ps[b][32 * s:32 * s + MW, :],
                        w1_sb[:, wofs:wofs + MW],
                        x_cb[ci][b][:, s * SUB:(s + 1) * SUB],
                        start=(i == 0),
                        stop=(i == 2 * NCH - 1),
                        tile_position=(0, 32 * s),
                        skip_group_check=True,
                    )

            def emit_sign(b):
                q_sb = qpool.tile([P, SUB], f16, tag="q", name=f"q{b}")
                nc.vector.tensor_scalar(
                    out=q_sb[:],
                    in0=z_ps[b][:],
                    scalar1=negb_sb[:],
                    scalar2=None,
                    op0=mybir.AluOpType.is_ge,
                )
                q_sbs[b] = q_sb
                o_sbs[b] = [
                    opool.tile([P, NSUB * D], f16, tag="osb", name=f"osb{b}_{s}")
                    for s in range(NSUB)
                ]

            def mm2_one(b, i):
                s, g = i // NSUB, i % NSUB
                q_sb = q_sbs[b]
                o_ps = opspool.tile([P, D], f32, tag="ops", name=f"ops{b}_{s}_{g}")
                nc.tensor.matmul(
                    o_ps[:],
                    q_sb[32 * s:32 * s + CA, g * P:(g + 1) * P],
                    w2_sb[32 * s:32 * s + CA, :],
                    start=True,
                    stop=True,
                    tile_position=(32 * s, 0),
                    skip_group_check=True,
                )
                # GpSimd cannot read PSUM: split the fp32->fp16 drain
                # copies between DVE and ScalarE (ScalarE one extra since
                # DVE also owns the sign op).
                dst = o_sbs[b][s][:, g * D:(g + 1) * D]
                if i % 2 == 0 or i == 15:
                    nc.scalar.copy(out=dst, in_=o_ps[:])
                else:
                    nc.vector.tensor_copy(out=dst, in_=o_ps[:])
                if g == NSUB - 1:
                    # Output DMAs ride the sync HWDGE queue behind the input
                    # loads. Measured better than the alternatives: the Act
                    # queue loads the ScalarE with issue costs it cannot
                    # spare, and the GpSimd software-DGE path is slower
                    # outright; the in-order trailing behind the (earlier,
                    # production-paced anyway) input stream costs less.
                    g0 = (b * BLK + s * SUB) // P
                    nc.sync.dma_start(
                        out=out[:, g0:g0 + NSUB, :],
                        in_=o_sbs[b][s][:],
                    )

            # Software pipeline: emit the whole mm1 of block b+1 before the
            # mm2 pairs of block b, so the PE rolls into the next block's
            # encoder while the DVE sign op for block b completes. (Finer
            # per-wave interleaving of mm1 and mm2 was measured WORSE —
            # alternating the two PE tile configurations kills the
            # column/row-strip stream packing.)
            for i in range(2 * NCH):
                mm1_wave(0, i)
            for b in range(NBLK):
                if b + 1 < NBLK:
                    for i in range(2 * NCH):
                        mm1_wave(b + 1, i)
                emit_sign(b)
                for i in range(NSUB * NSUB):
                    mm2_one(b, i)
    nc.compile()
    return nc


def _get_nc():
    if "nc" not in _CACHE:
        _CACHE["nc"] = _build_nc()
    return _CACHE["nc"]


def _prep_weights(W_enc, b_enc, W_dec, b_dec):
    f16, f32 = np.float16, np.float32
    WT = np.ascontiguousarray(W_enc.T.astype(f32))            # [512, 16]
    Wh = WT.astype(f16)
    Wl = (WT - Wh.astype(f32)).astype(f16)
    # 8 lhsT tiles of [128, 32]: (Wh, Wl) per K-chunk, cols 16..31 = 0 so
    # every z row is written (row 16 = 0 feeds the bias trick, 17..31 junk)
    w1 = np.zeros((P, NW1), f16)
    for p, src in enumerate((Wh, Wl)):
        for c in range(NCH):
            ofs = (p * NCH + c) * MW
            w1[:, ofs:ofs + C] = src[c * P:(c + 1) * P, :]

    # w2: replica of [2*W_dec.T ; bias_row] in each 32-row band; nb: the
    # per-partition sign thresholds (-b_enc on the 16 real rows, -1
    # elsewhere: the zero z bias-row maps to q=1, rows 17..31 are unread).
    w2 = np.zeros((P, D), f16)
    band = np.concatenate(
        [2.0 * W_dec.T.astype(f32),
         (b_dec.astype(f32) - W_dec.astype(f32).sum(axis=1)).reshape(1, D)],
        axis=0,
    ).astype(f16)                                             # [17, 512]
    negb = np.full((P, 1), -1.0, f32)
    for s in range(NSUB):
        w2[32 * s:32 * s + CA, :] = band
        negb[32 * s:32 * s + C, 0] = -b_enc.astype(f32)
    return w1, w2, negb


def _prep_x_shard(x_flat_shard):
    """[8192, 512] fp32 -> [4, 128, 8192] fp16 feature-major (chunk, part, tok)."""
    xh = x_flat_shard.astype(np.float16)
    return np.ascontiguousarray(xh.T).reshape(NCH, P, TOK)


def kernel(x, W_enc, b_enc, W_dec, b_dec, _trace=False, _trace_kwargs=None):
    x = np.asarray(x, dtype=np.float32)
    w1, w2, nb = _prep_weights(
        np.asarray(W_enc), np.asarray(b_enc), np.asarray(W_dec), np.asarray(b_dec)
    )
    xf = x.reshape(NCORES, TOK, D)
    in_maps = []
    for s in range(NCORES):
        in_maps.append(dict(xt=_prep_x_shard(xf[s]), w1=w1, w2=w2, nb=nb))
    nc = _get_nc()
    res = run_bass_kernel_spmd(
        nc,
        in_maps,
        core_ids=list(range(NCORES)),
        trace=_trace,
        **(_trace_kwargs or {}),
    )
    out = np.concatenate(
        [
            res.results[s]["out"].transpose(1, 0, 2).reshape(1, TOK, D)
            for s in range(NCORES)
        ],
        axis=0,
    ).astype(np.float32).reshape(B, H, W_, D)
    _CACHE["last_results"] = res
    return out
